# revision 1
# baseline (speedup 1.0000x reference)
"""Trainium2 Bass kernel for DeepME edge-MLP (gnn_message_passing), v2.

Contract: kernel(**inputs) takes FULL unsharded inputs and returns the FULL
[E, 1] float32 output.  Data-parallel over edges across 8 NeuronCores.

v2 design (vs the feature-major v1 baseline):
  - Edges sorted by src, split per core, each core's half-groups re-sorted
    by dst.  Both src and dst embedding gathers then fit int16 index
    windows, so both use dma_gather(transpose=True): rows arrive
    FEATURE-major in SBUF directly (no gather transposes, no psum round
    trip), from a 256-column padded fp16 table whose column 192 is 1.0 --
    supplying the K-dim "ones" row that turns layer biases into an extra
    weight row.
  - All matmuls run in the edge-major-output orientation: out[edge, feat]
    with lhsT = activationT (features on partitions) and rhs = weights.
    PE cost = out free size = n_features << n_edges.  LayerNorm statistics
    become per-partition (per-edge) quantities: bn_stats + tensor_scalar
    with [P,1] scalar APs, no transposes around LN, no rank-k mean
    corrections.
  - fp16 data everywhere on PE (1 cyc/row matmuls+transposes), f32 psums.
"""

import numpy as np

# ---------------------------------------------------------------------------
E_TOTAL = 300000
N_NODES = 300000
H = 192
NCORES = 8
P = 128
CH = 4
TILE = P * CH               # 512 edges per tile
E_PC = E_TOTAL // NCORES    # 37500
NT = (E_PC + TILE - 1) // TILE   # 74 tiles per core
EC = 256                    # padded embedding row (fp16, 512B; col 192 = 1.0)
LN_EPS = 1e-5

_PROG_CACHE = {}


def _build_program(repeat=1):
    from contextlib import ExitStack

    import concourse.bass as bass
    import concourse.bacc as bacc
    import concourse.tile as tile
    import concourse.mybir as mybir

    dt = mybir.dt
    f32 = dt.float32
    f16 = dt.float16
    i32 = dt.int32
    AF = mybir.ActivationFunctionType
    OP = mybir.AluOpType
    AX = mybir.AxisListType

    nc = bacc.Bacc(trn_type="TRN2", target_bir_lowering=False, debug=False,
                   num_devices=NCORES)

    emb_d = nc.dram_tensor("emb", [N_NODES, EC], f16, kind="ExternalInput").ap()
    sidx_d = nc.dram_tensor("sidx", [P, NT, CH], i32, kind="ExternalInput").ap()
    didx_d = nc.dram_tensor("didx", [P, NT, CH], i32, kind="ExternalInput").ap()
    etf_d = nc.dram_tensor("etf", [P, NT, CH], f32, kind="ExternalInput").ap()
    wpa_d = nc.dram_tensor("wpa", [P, 1539], f16, kind="ExternalInput").ap()
    wpb_d = nc.dram_tensor("wpb", [P, 963], f16, kind="ExternalInput").ap()
    idn_d = nc.dram_tensor("idn", [P, P], f16, kind="ExternalInput").ap()
    cst_d = nc.dram_tensor("cst", [P, 33], f32, kind="ExternalInput").ap()
    out_d = nc.dram_tensor("out", [P, NT, CH], f32, kind="ExternalOutput").ap()

    def mm(out, lhsT, rhs, start, stop):
        nc.tensor.matmul(out=out, lhsT=lhsT, rhs=rhs, start=start, stop=stop)

    BR = ((0, 192), (192, 384), (384, 448), (448, 512), (512, 576))

    with tile.TileContext(nc) as tc, ExitStack() as ctx:
        cpool = ctx.enter_context(tc.tile_pool(name="const", bufs=1))
        sb = ctx.enter_context(tc.tile_pool(name="work", bufs=1))
        pp = ctx.enter_context(tc.tile_pool(name="psum", bufs=1, space="PSUM"))

        # ---- resident tiles ------------------------------------------------
        sidx = cpool.tile([P, NT, CH], i32)
        didx = cpool.tile([P, NT, CH], i32)
        etf = cpool.tile([P, NT, CH], f32)
        outp = cpool.tile([P, NT, CH], f32)
        wpa = cpool.tile([P, 1539], f16)
        wpb = cpool.tile([P, 963], f16)
        idn = cpool.tile([P, P], f16)
        cst = cpool.tile([P, 33], f32)
        nc.sync.dma_start(sidx[:], sidx_d[:])
        nc.sync.dma_start(didx[:], didx_d[:])
        nc.sync.dma_start(etf[:], etf_d[:])
        nc.sync.dma_start(wpa[:], wpa_d[:])
        nc.sync.dma_start(wpb[:], wpb_d[:])
        nc.sync.dma_start(idn[:], idn_d[:])
        nc.sync.dma_start(cst[:], cst_d[:])

        iota3 = cst[:, 0:12].rearrange("p (c t) -> p c t", t=3)
        epsc = cst[:, 12:13]
        c4n = cst[:, 13:33].rearrange("p (c b) -> p c b", b=5)

        # weight slices (A rows 0:128, B rows 0:65 incl bias row 64)
        wsA, wsB = wpa[:, 0:192], wpb[0:65, 0:192]
        wdA, wdB = wpa[:, 192:384], wpb[0:65, 192:384]
        w1A, w1B = wpa[:, 384:448], wpb[0:65, 384:448]
        w2A, w2B = wpa[:, 448:512], wpb[0:65, 448:512]
        w3A, w3B = wpa[:, 512:576], wpb[0:65, 512:576]
        wf1 = [wpa[:, 576 + 192 * k: 768 + 192 * k] for k in range(4)]
        wf1B = wpb[0:65, 576:768]
        wf2A, wf2B = wpa[:, 1344:1536], wpb[0:65, 768:960]
        wf3A, wf3B = wpa[:, 1536:1539], wpb[0:65, 960:963]

        def tp(out, in_):
            k = in_.partition_size()
            nc.tensor.transpose(out=out, in_=in_, identity=idn[0:k, 0:k])

        # "ones row" tiles: row 64 preset to 1.0, rewritten rows are 0:64 only
        def ones_tile(name, bufs=2):
            ts = []
            for i in range(bufs):
                t = sb.tile([65, TILE], f16, tag=name, bufs=bufs,
                            name=f"{name}{i}")
                nc.vector.memset(t[64:65, :], 1.0)
                ts.append(t)
            return ts

        ones_tile("difB")
        ones_tile("prdB")
        ones_tile("sqB")
        ones_tile("y2s1")
        ones_tile("r2s1")
        # merged ysb: bias ones row = partition 64 of k-slot 4
        for i in range(2):
            t = sb.tile([P, 5, TILE], f16, tag="ysb", bufs=2, name=f"ysbi{i}")
            nc.vector.memset(t[64:65, 4, :], 1.0)

        # ---- PSUM: exactly 8 banks, manually packed ------------------------
        # srcTp/dstTp: gather-transpose psums [A | B] halves
        # bpA0/bpA1: branch psum, chunk-parity double buffer
        # sml: [bpB p0 | bpB p1 | fp slot0 | fp slot1] f32 columns
        # ypP: fusion-transpose psum, 5 k-slots per chunk (single buffer)
        # upP: slot0 = up0 (128p), slot1 rows 0:64 = up1
        # l3s: logits psum, tile-parity in dim 1
        srcTp = pp.tile([P, 2, TILE], f16, tag="srcTp", name="srcTp")
        dstTp = pp.tile([P, 2, TILE], f16, tag="dstTp", name="dstTp")
        bpA = [pp.tile([P, 512], f32, tag="bpA0", name="bpA0"),
               pp.tile([P, 512], f32, tag="bpA1", name="bpA1")]
        sml = pp.tile([P, 512], f32, tag="sml", name="sml")
        ypP = pp.tile([P, 5, P], f16, tag="ypP", name="ypP")
        upP = pp.tile([P, 2, TILE], f16, tag="upP", name="upP")
        up0 = upP[:, 0, :]
        up1 = upP[:, 1, :]
        l3s = pp.tile([P, 2, CH, 3], f32, tag="l3s", name="l3s")

        # ---- per-tile phases ----------------------------------------------
        def gather(tau):
            srcE = sb.tile([P, CH, EC], f16, tag="srcE", bufs=3, name="srcE")
            dstE = sb.tile([P, CH, EC], f16, tag="dstE", bufs=3, name="dstE")
            for c in range(CH):
                nc.gpsimd.indirect_dma_start(
                    out=srcE[:, c, :], out_offset=None, in_=emb_d[:, :],
                    in_offset=bass.IndirectOffsetOnAxis(
                        ap=sidx[:, tau, c:c + 1], axis=0))
                nc.gpsimd.indirect_dma_start(
                    out=dstE[:, c, :], out_offset=None, in_=emb_d[:, :],
                    in_offset=bass.IndirectOffsetOnAxis(
                        ap=didx[:, tau, c:c + 1], axis=0))
            return srcE, dstE

        def phase1(srcE, dstE):
            # transpose gathered rows to feature-major [A | B] halves; the
            # table's 1.0 column lands on partition 64 of the B half.
            srcT = sb.tile([P, 2, TILE], f16, tag="srcT", bufs=2, name="srcT")
            dstT = sb.tile([P, 2, TILE], f16, tag="dstT", bufs=2, name="dstT")
            for c in range(CH):
                cs = slice(c * P, (c + 1) * P)
                tp(srcTp[:, 0, cs], srcE[:, c, 0:128])
                tp(srcTp[:, 1, cs], srcE[:, c, 128:256])
                tp(dstTp[:, 0, cs], dstE[:, c, 0:128])
                tp(dstTp[:, 1, cs], dstE[:, c, 128:256])
            nc.any.tensor_copy(srcT[:], srcTp[:, :, :])
            nc.any.tensor_copy(dstT[:], dstTp[:, :, :])

            difA = sb.tile([P, TILE], f16, tag="difA", bufs=2, name="difA")
            prdA = sb.tile([P, TILE], f16, tag="prdA", bufs=2, name="prdA")
            sqA = sb.tile([P, TILE], f16, tag="sqA", bufs=2, name="sqA")
            difB = sb.tile([65, TILE], f16, tag="difB", bufs=2, name="difB")
            prdB = sb.tile([65, TILE], f16, tag="prdB", bufs=2, name="prdB")
            sqB = sb.tile([65, TILE], f16, tag="sqB", bufs=2, name="sqB")
            nc.vector.tensor_sub(difA[:], srcT[:, 0, :], dstT[:, 0, :])
            nc.vector.tensor_sub(difB[0:64, :], srcT[0:64, 1, :], dstT[0:64, 1, :])
            nc.vector.tensor_mul(prdA[:], srcT[:, 0, :], dstT[:, 0, :])
            nc.vector.tensor_mul(prdB[0:64, :], srcT[0:64, 1, :], dstT[0:64, 1, :])
            nc.scalar.activation(sqA[:], difA[:], AF.Square)
            nc.scalar.activation(sqB[0:64, :], difB[0:64, :], AF.Square)

            h = sb.tile([P, CH, 576], f16, tag="h", bufs=2, name="h")
            for c in range(CH):
                cs = slice(c * P, (c + 1) * P)
                bp = bpA[c % 2]
                bpB = sml[:, (c % 2) * 64:(c % 2) * 64 + 64]
                mm(bp[:, 0:192], srcT[:, 0, cs], wsA, True, False)
                mm(bp[:, 0:192], srcT[0:65, 1, cs], wsB, False, True)
                mm(bp[:, 192:384], dstT[:, 0, cs], wdA, True, False)
                mm(bp[:, 192:384], dstT[0:65, 1, cs], wdB, False, True)
                mm(bp[:, 384:448], difA[:, cs], w1A, True, False)
                mm(bp[:, 384:448], difB[:, cs], w1B, False, True)
                mm(bp[:, 448:512], sqA[:, cs], w2A, True, False)
                mm(bp[:, 448:512], sqB[:, cs], w2B, False, True)
                mm(bpB[:, 0:64], prdA[:, cs], w3A, True, False)
                mm(bpB[:, 0:64], prdB[:, cs], w3B, False, True)
                nc.scalar.activation(h[:, c, 0:512], bp[:, :], AF.Relu)
                nc.scalar.activation(h[:, c, 512:576], bpB[:, 0:64], AF.Relu)

            # LN stats: sum / sum-of-squares via multi-group reduces, then
            # y = h*istd - (mu*istd) with free-dim broadcast views.
            hsq = sb.tile([P, CH, 576], f16, tag="hsq", bufs=2, name="hsq")
            nc.vector.tensor_mul(hsq[:], h[:], h[:])
            su = sb.tile([P, CH, 5], f32, tag="su", bufs=2, name="su")
            qu = sb.tile([P, CH, 5], f32, tag="qu", bufs=2, name="qu")
            for src_, dst_ in ((h, su), (hsq, qu)):
                v2 = src_[:, :, 0:384].rearrange("p c (b f) -> p c b f", b=2)
                v3 = src_[:, :, 384:576].rearrange("p c (b f) -> p c b f", b=3)
                nc.vector.tensor_reduce(out=dst_[:, :, 0:2], in_=v2,
                                        axis=AX.X, op=OP.add)
                nc.vector.tensor_reduce(out=dst_[:, :, 2:5], in_=v3,
                                        axis=AX.X, op=OP.add)
            mu_t = sb.tile([P, CH, 5], f32, tag="mu_t", bufs=2, name="mu_t")
            ms_t = sb.tile([P, CH, 5], f32, tag="ms_t", bufs=2, name="ms_t")
            t_t = sb.tile([P, CH, 5], f32, tag="t_t", bufs=2, name="t_t")
            is_t = sb.tile([P, CH, 5], f32, tag="is_t", bufs=2, name="is_t")
            q_t = sb.tile([P, CH, 5], f32, tag="q_t", bufs=2, name="q_t")
            nc.vector.tensor_mul(mu_t[:], su[:], c4n)
            nc.vector.tensor_mul(ms_t[:], qu[:], c4n)
            nc.vector.scalar_tensor_tensor(
                out=t_t[:], in0=mu_t[:], scalar=1.0, in1=mu_t[:],
                op0=OP.mult, op1=OP.mult)
            nc.vector.tensor_sub(ms_t[:], ms_t[:], t_t[:])
            nc.scalar.activation(t_t[:], ms_t[:], AF.Ln, bias=epsc)
            nc.scalar.activation(is_t[:], t_t[:], AF.Exp, scale=-0.5)
            nc.vector.scalar_tensor_tensor(
                out=q_t[:], in0=mu_t[:], scalar=1.0, in1=is_t[:],
                op0=OP.mult, op1=OP.mult)

            y = sb.tile([P, CH, 576], f16, tag="y", bufs=2, name="y")
            h1 = h[:, :, 0:384].rearrange("p c (b f) -> p c b f", b=2)
            h2 = h[:, :, 384:576].rearrange("p c (b f) -> p c b f", b=3)
            y1 = y[:, :, 0:384].rearrange("p c (b f) -> p c b f", b=2)
            y2v = y[:, :, 384:576].rearrange("p c (b f) -> p c b f", b=3)
            i1 = is_t[:, :, 0:2].unsqueeze(3).to_broadcast([P, CH, 2, 192])
            i2 = is_t[:, :, 2:5].unsqueeze(3).to_broadcast([P, CH, 3, 64])
            q1 = q_t[:, :, 0:2].unsqueeze(3).to_broadcast([P, CH, 2, 192])
            q2 = q_t[:, :, 2:5].unsqueeze(3).to_broadcast([P, CH, 3, 64])
            nc.vector.tensor_mul(y1, h1, i1)
            nc.vector.tensor_mul(y2v, h2, i2)
            nc.vector.tensor_sub(y1, y1, q1)
            nc.vector.tensor_sub(y2v, y2v, q2)
            return y

        def phase2a(y):
            # transpose y up to feature-major K-chunks: one psum bank holds
            # the 5 k-slots of one chunk; ysb is a single [P, 5, TILE] tile.
            ysb = sb.tile([P, 5, TILE], f16, tag="ysb", bufs=2, name="ysb")
            for c in range(CH):
                cs = slice(c * P, (c + 1) * P)
                for k in range(4):
                    tp(ypP[:, k, :], y[:, c, k * P:(k + 1) * P])
                tp(ypP[0:64, 4, :], y[:, c, 512:576])
                nc.any.tensor_copy(ysb[:, 0:4, cs], ypP[:, 0:4, :])
                nc.any.tensor_copy(ysb[0:64, 4, cs], ypP[0:64, 4, :])

            rf = sb.tile([P, CH, 192], f16, tag="rf", bufs=2, name="rf")
            for c in range(CH):
                cs = slice(c * P, (c + 1) * P)
                fp = sml[:, 128 + (c % 2) * 192:320 + (c % 2) * 192]
                for k in range(4):
                    mm(fp, ysb[:, k, cs], wf1[k], k == 0, False)
                mm(fp, ysb[0:65, 4, cs], wf1B, False, True)
                nc.scalar.activation(rf[:, c, :], fp, AF.Relu)

            # fusion LN, reduce-based stats + broadcast scale
            rfsq = sb.tile([P, CH, 192], f16, tag="rfsq", bufs=2, name="rfsq")
            nc.vector.tensor_mul(rfsq[:], rf[:], rf[:])
            suf = sb.tile([P, CH], f32, tag="suf", bufs=2, name="suf")
            quf = sb.tile([P, CH], f32, tag="quf", bufs=2, name="quf")
            nc.vector.tensor_reduce(out=suf[:], in_=rf[:], axis=AX.X, op=OP.add)
            nc.vector.tensor_reduce(out=quf[:], in_=rfsq[:], axis=AX.X, op=OP.add)
            muf = sb.tile([P, CH], f32, tag="muf", bufs=2, name="muf")
            msf = sb.tile([P, CH], f32, tag="msf", bufs=2, name="msf")
            ttf = sb.tile([P, CH], f32, tag="ttf", bufs=2, name="ttf")
            isf = sb.tile([P, CH], f32, tag="isf", bufs=2, name="isf")
            qf = sb.tile([P, CH], f32, tag="qf", bufs=2, name="qf")
            nc.vector.tensor_scalar(out=muf[:], in0=suf[:], scalar1=1.0 / H,
                                    scalar2=None, op0=OP.mult)
            nc.vector.tensor_scalar(out=msf[:], in0=quf[:], scalar1=1.0 / H,
                                    scalar2=None, op0=OP.mult)
            nc.vector.scalar_tensor_tensor(
                out=ttf[:], in0=muf[:], scalar=1.0, in1=muf[:],
                op0=OP.mult, op1=OP.mult)
            nc.vector.tensor_sub(msf[:], msf[:], ttf[:])
            nc.scalar.activation(ttf[:], msf[:], AF.Ln, bias=epsc)
            nc.scalar.activation(isf[:], ttf[:], AF.Exp, scale=-0.5)
            nc.vector.scalar_tensor_tensor(
                out=qf[:], in0=muf[:], scalar=1.0, in1=isf[:],
                op0=OP.mult, op1=OP.mult)
            y2 = sb.tile([P, CH, 192], f16, tag="y2", bufs=2, name="y2")
            isfb = isf[:].unsqueeze(2).to_broadcast([P, CH, H])
            qfb = qf[:].unsqueeze(2).to_broadcast([P, CH, H])
            nc.vector.tensor_mul(y2[:], rf[:], isfb)
            nc.vector.tensor_sub(y2[:], y2[:], qfb)
            return y2

        def phase2b(y2, tau):
            y2s0 = sb.tile([P, TILE], f16, tag="y2s0", bufs=2, name="y2s0")
            y2s1 = sb.tile([65, TILE], f16, tag="y2s1", bufs=2, name="y2s1")
            for c in range(CH):
                cs = slice(c * P, (c + 1) * P)
                tp(up0[:, cs], y2[:, c, 0:128])
                tp(up1[0:64, cs], y2[:, c, 128:192])
            nc.any.tensor_copy(y2s0[:], up0[:, :])
            nc.any.tensor_copy(y2s1[0:64, :], up1[0:64, :])

            r2 = sb.tile([P, CH, 192], f16, tag="r2", bufs=2, name="r2")
            for c in range(CH):
                cs = slice(c * P, (c + 1) * P)
                f2 = sml[:, 128 + (c % 2) * 192:320 + (c % 2) * 192]
                mm(f2, y2s0[:, cs], wf2A, True, False)
                mm(f2, y2s1[:, cs], wf2B, False, True)
                nc.scalar.activation(r2[:, c, :], f2, AF.Relu)

            r2s0 = sb.tile([P, TILE], f16, tag="r2s0", bufs=2, name="r2s0")
            r2s1 = sb.tile([65, TILE], f16, tag="r2s1", bufs=2, name="r2s1")
            for c in range(CH):
                cs = slice(c * P, (c + 1) * P)
                tp(up0[:, cs], r2[:, c, 0:128])
                tp(up1[0:64, cs], r2[:, c, 128:192])
            nc.any.tensor_copy(r2s0[:], up0[:, :])
            nc.any.tensor_copy(r2s1[0:64, :], up1[0:64, :])

            l3 = l3s[:, tau % 2, :, :]
            for c in range(CH):
                cs = slice(c * P, (c + 1) * P)
                mm(l3[:, c, :], r2s0[:, cs], wf3A, True, False)
                mm(l3[:, c, :], r2s1[:, cs], wf3B, False, True)

            l_e = sb.tile([P, CH, 3], f32, tag="l_e", bufs=2, name="l_e")
            nc.scalar.activation(l_e[:], l3[:, :, :], AF.Copy)
            oh = sb.tile([P, CH, 3], f32, tag="oh", bufs=2, name="oh")
            etb = etf[:, tau, :].unsqueeze(2).to_broadcast([P, CH, 3])
            nc.vector.tensor_tensor(out=oh[:], in0=etb, in1=iota3,
                                    op=OP.is_equal)
            nc.vector.tensor_mul(oh[:], oh[:], l_e[:])
            sel = sb.tile([P, CH], f32, tag="sel", bufs=2, name="sel")
            nc.vector.tensor_reduce(out=sel[:], in_=oh[:], axis=AX.X, op=OP.add)
            selb = sel[:].unsqueeze(2).to_broadcast([P, CH, 3])
            nc.vector.tensor_sub(l_e[:], l_e[:], selb)
            ex = sb.tile([P, CH, 3], f32, tag="ex", bufs=2, name="ex")
            nc.scalar.activation(ex[:], l_e[:], AF.Exp)
            den = sb.tile([P, CH], f32, tag="den", bufs=2, name="den")
            nc.vector.tensor_reduce(out=den[:], in_=ex[:], axis=AX.X, op=OP.add)
            nc.vector.reciprocal(outp[:, tau, :], den[:])

        def whole_body(_iv=None):
            gs, ys, y2s = {}, {}, {}
            gs[0] = gather(0)
            if NT > 1:
                gs[1] = gather(1)
            for i in range(NT + 2):
                if i + 2 < NT:
                    gs[i + 2] = gather(i + 2)
                if i < NT:
                    ys[i] = phase1(*gs.pop(i))
                if 0 <= i - 1 < NT:
                    y2s[i - 1] = phase2a(ys.pop(i - 1))
                if i - 2 >= 0:
                    phase2b(y2s.pop(i - 2), i - 2)

        if repeat > 1:
            with tc.For_i(0, repeat, 1):
                whole_body()
        else:
            whole_body()

        nc.sync.dma_start(out_d[:], outp[:])

    # Pin the ACT table set (Relu/Square/Ln/Exp/Copy/Identity all live in
    # natural_log_exp_and_others) so no table reloads occur.
    import concourse.bacc as _bacc_mod
    _orig_gat = _bacc_mod.get_activation_tables

    def _pinned_tables(arch):
        tabs = _orig_gat(arch)
        return {name: (s if name == "natural_log_exp_and_others" else set())
                for name, s in tabs.items()}

    _bacc_mod.get_activation_tables = _pinned_tables
    try:
        nc.compile()
    finally:
        _bacc_mod.get_activation_tables = _orig_gat
    return nc


def _get_program(repeat=1):
    key = repeat
    if key not in _PROG_CACHE:
        _PROG_CACHE[key] = _build_program(repeat)
    return _PROG_CACHE[key]


_SCATTER = {"order": None}


def _host_prep(inputs):
    f = lambda k: np.asarray(inputs[k], np.float32)
    kge = f("kge_emb")
    ei = np.asarray(inputs["edge_index"]).astype(np.int64)
    et = np.asarray(inputs["edge_type"]).astype(np.int64)

    W1, b1, g1, be1 = f("W1"), f("b1"), f("g1"), f("be1")
    W2, b2, g2, be2 = f("W2"), f("b2"), f("g2"), f("be2")
    W3, b3, g3, be3 = f("W3"), f("b3"), f("g3"), f("be3")
    Ws, bs, gs_, bes = f("Ws"), f("bs"), f("gs"), f("bes")
    Wd, bd, gd, bed = f("Wd"), f("bd"), f("gd"), f("bed")
    Wf1, bf1, gf, bef = f("Wf1"), f("bf1"), f("gf"), f("bef")
    Wf2, bf2 = f("Wf2"), f("bf2")
    Wf3, bf3 = f("Wf3"), f("bf3")

    g_cat = np.concatenate([gs_, gd, g1, g2, g3])
    be_cat = np.concatenate([bes, bed, be1, be2, be3])
    Wf1_eff = g_cat[:, None] * Wf1
    bf1_eff = bf1 + be_cat @ Wf1
    Wf2_eff = gf[:, None] * Wf2
    bf2_eff = bf2 + bef @ Wf2

    # packed weights, fp16, bias folded as row 64 of the B chunks
    wpa = np.zeros((P, 1539), np.float32)
    wpb = np.zeros((P, 963), np.float32)

    def put(Wm, bias, a_sl, b_sl):
        wpa[:, a_sl] = Wm[0:128]
        wpb[0:64, b_sl] = Wm[128:192]
        wpb[64, b_sl] = bias

    put(Ws, bs, slice(0, 192), slice(0, 192))
    put(Wd, bd, slice(192, 384), slice(192, 384))
    put(W1, b1, slice(384, 448), slice(384, 448))
    put(W2, b2, slice(448, 512), slice(448, 512))
    put(W3, b3, slice(512, 576), slice(512, 576))
    for k in range(4):
        wpa[:, 576 + 192 * k: 768 + 192 * k] = Wf1_eff[128 * k:128 * (k + 1)]
    wpb[0:64, 576:768] = Wf1_eff[512:576]
    wpb[64, 576:768] = bf1_eff
    put(Wf2_eff, bf2_eff, slice(1344, 1536), slice(768, 960))
    wpa[:, 1536:1539] = Wf3[0:128]
    wpb[0:64, 960:963] = Wf3[128:192]
    wpb[64, 960:963] = bf3
    wpa16 = wpa.astype(np.float16)
    wpb16 = wpb.astype(np.float16)

    idn = np.eye(P, dtype=np.float16)
    cst = np.zeros((P, 33), np.float32)
    cst[:, 0:12] = np.tile(np.arange(3, dtype=np.float32), CH)[None, :]
    cst[:, 12] = LN_EPS
    c4n = np.array([1.0 / H, 1.0 / H, 1.0 / 64, 1.0 / 64, 1.0 / 64],
                   np.float32)
    cst[:, 13:33] = np.tile(c4n, CH)[None, :]

    # padded fp16 table: col 192 = 1.0 (bias ones row source)
    ptab = np.zeros((N_NODES, EC), np.float16)
    ptab[:, 0:H] = kge.astype(np.float16)
    ptab[:, H] = 1.0

    # ---- edge ordering: sort by src for gather locality -------------------
    order0 = np.argsort(ei[0], kind="stable")
    scatter = np.full((NCORES, NT * TILE), -1, np.int64)
    in_maps = []
    shared = dict(emb=ptab, wpa=wpa16, wpb=wpb16, idn=idn, cst=cst)

    def arrange(a, dtype):
        buf = np.zeros(NT * TILE, dtype)
        buf[:E_PC] = a
        return np.ascontiguousarray(
            buf.reshape(NT, CH, P).transpose(2, 0, 1))

    for core in range(NCORES):
        blk = order0[core * E_PC:(core + 1) * E_PC]
        m = dict(shared)
        m["sidx"] = arrange(ei[0][blk].astype(np.int32), np.int32)
        m["didx"] = arrange(ei[1][blk].astype(np.int32), np.int32)
        m["etf"] = arrange(et[blk].astype(np.float32), np.float32)
        pos = np.full(NT * TILE, -1, np.int64)
        pos[:E_PC] = blk
        scatter[core] = pos
        in_maps.append(m)
    _SCATTER["order"] = scatter
    return in_maps


def _unshard(results):
    out = np.zeros(E_TOTAL, np.float32)
    scatter = _SCATTER["order"]
    for core in range(NCORES):
        o = np.asarray(results[core]["out"], np.float32)  # [P, NT, CH]
        # local edge i = tau*512 + c*128 + p  ->  o[p, tau, c]
        flat = o.transpose(1, 2, 0).reshape(-1)
        pos = scatter[core]
        valid = pos >= 0
        out[pos[valid]] = flat[valid]
    return out[:, None].astype(np.float32)


def kernel(**inputs):
    from concourse.bass_utils import run_bass_kernel_spmd
    nc = _get_program()
    in_maps = _host_prep(inputs)
    res = run_bass_kernel_spmd(nc, in_maps, list(range(NCORES)))
    return _unshard(res.results)

